# revision 7
# baseline (speedup 1.0000x reference)
"""Trainium2 Bass kernel for NeuralODEMemory (nn_NeuralODEMemory_28355374088720).

Math (reference):
    dt = 0.1, 10 Euler steps over h (N=65536 rows, D=512):
        z = [h, t]                              # time feature column
        deriv = tanh(tanh(z @ W1.T + b1) @ W2.T + b2)
        h <- h + dt * deriv
    gate  = sigmoid([x, h10] @ Wg.T + bg)
    out   = gate * h10 + (1 - gate) * x

Kernel strategy:
  * Data-parallel over 8 NeuronCores (8192 rows each); weights replicated.
  * Feature-major ("transposed") activation layout [D, rows] on chip so the
    per-step matmul chain needs no transposes: weights are the stationary
    operand ([in,out] chunks), activations stream as the moving operand, and
    each layer's PSUM output is already in the layout the next layer consumes.
  * The time-feature column is algebraically folded into a per-step bias:
    z @ W1.T = h @ W1[:, :D].T + t * W1[:, D], so b1_eff(s) = b1 + t_s*W1[:,D].
    Bias-add is free via the ACT engine's activation(out = f(in*scale + bias)).
  * h is kept "primed" as h' = h/dt with W1/WgB pre-scaled by dt on the host,
    so the per-step update is a single tensor_tensor add: h' += deriv.
  * bf16 matmuls (fp32 PSUM accumulation), fp32 h accumulator, fp32 x/out.
  * Host does the cheap prep: weight transpose/scale/cast, x/h transposes.
"""

import os
from contextlib import ExitStack

import numpy as np
import ml_dtypes

N_TOTAL = 65536
D = 512
NCORES = 8
NPC = N_TOTAL // NCORES          # rows per core
NUM_STEPS = 10
TIME_INTERVAL = 1.0
DT = TIME_INTERVAL / NUM_STEPS
P = 128
FK = D // P                      # feature chunks of 128 (4)
MMN = 512                        # matmul moving-operand free dim (one PSUM bank)

_CACHE = {}
LAST = {}                        # stash of the last run's BassKernelResults


def _build(npc, rblk=1024):
    import concourse.mybir as mybir
    import concourse.tile as tile
    from concourse import bacc

    f32 = mybir.dt.float32
    bf16 = mybir.dt.bfloat16
    Act = mybir.ActivationFunctionType
    Alu = mybir.AluOpType

    nblk = npc // rblk
    nsub = rblk // MMN

    nc = bacc.Bacc("TRN2", target_bir_lowering=False, debug=False,
                   num_devices=NCORES)

    hT = nc.dram_tensor("hT", [D, npc], f32, kind="ExternalInput").ap()
    xT = nc.dram_tensor("xT", [D, npc], f32, kind="ExternalInput").ap()
    xTb = nc.dram_tensor("xTb", [D, npc], bf16, kind="ExternalInput").ap()
    w1t = nc.dram_tensor("w1t", [D, D], bf16, kind="ExternalInput").ap()
    w2t = nc.dram_tensor("w2t", [D, D], bf16, kind="ExternalInput").ap()
    wgt = nc.dram_tensor("wgt", [2 * D, D], bf16, kind="ExternalInput").ap()
    b1e = nc.dram_tensor("b1e", [P, NUM_STEPS * FK], f32, kind="ExternalInput").ap()
    b2c = nc.dram_tensor("b2c", [P, FK], f32, kind="ExternalInput").ap()
    bgc = nc.dram_tensor("bgc", [P, FK], f32, kind="ExternalInput").ap()
    outT = nc.dram_tensor("outT", [D, npc], f32, kind="ExternalOutput").ap()

    hTr = hT.rearrange("(k p) r -> p k r", p=P)
    xTr = xT.rearrange("(k p) r -> p k r", p=P)
    xTbr = xTb.rearrange("(k p) r -> p k r", p=P)
    outTr = outT.rearrange("(k p) r -> p k r", p=P)

    with tile.TileContext(nc) as tc:
        with ExitStack() as ctx:
            consts = ctx.enter_context(tc.tile_pool(name="consts", bufs=1))
            hp = ctx.enter_context(tc.tile_pool(name="h", bufs=3))
            hbp = ctx.enter_context(tc.tile_pool(name="hb", bufs=2))
            apool = ctx.enter_context(tc.tile_pool(name="a", bufs=2))
            dpool = ctx.enter_context(tc.tile_pool(name="d", bufs=2))
            xbp = ctx.enter_context(tc.tile_pool(name="xb", bufs=2))
            xfp = ctx.enter_context(tc.tile_pool(name="xf", bufs=2))
            gp = ctx.enter_context(tc.tile_pool(name="g", bufs=2))
            scp = ctx.enter_context(tc.tile_pool(name="sc", bufs=6))
            psp = ctx.enter_context(tc.tile_pool(name="ps", bufs=4, space="PSUM"))

            # Replicated constants, resident for the whole kernel.
            w1 = consts.tile([P, FK, D], bf16)
            nc.sync.dma_start(w1[:], w1t.rearrange("(k p) m -> p k m", p=P))
            w2 = consts.tile([P, FK, D], bf16)
            nc.sync.dma_start(w2[:], w2t.rearrange("(k p) m -> p k m", p=P))
            wg = consts.tile([P, 2 * FK, D], bf16)
            nc.sync.dma_start(wg[:], wgt.rearrange("(k p) m -> p k m", p=P))
            b1 = consts.tile([P, NUM_STEPS * FK], f32)
            nc.sync.dma_start(b1[:], b1e)
            b2 = consts.tile([P, FK], f32)
            nc.sync.dma_start(b2[:], b2c)
            bg = consts.tile([P, FK], f32)
            nc.sync.dma_start(bg[:], bgc)

            def mm_group(ps_tile, wtile, wk, m, rhs_tile, rk, start, stop):
                # one [128,128] stationary chunk x nsub moving tiles of MMN rows
                for sub in range(nsub):
                    nc.tensor.matmul(
                        ps_tile[:, sub * MMN:(sub + 1) * MMN],
                        wtile[:, wk, m * P:(m + 1) * P],
                        rhs_tile[:, rk, sub * MMN:(sub + 1) * MMN],
                        start=start, stop=stop,
                    )

            for blk in range(nblk):
                rs = blk * rblk
                h = hp.tile([P, FK, rblk], f32, tag="h")
                nc.sync.dma_start(h[:], hTr[:, :, rs:rs + rblk])
                xb = xbp.tile([P, FK, rblk], bf16, tag="xb")
                nc.sync.dma_start(xb[:], xTbr[:, :, rs:rs + rblk])
                xf = xfp.tile([P, FK, rblk], f32, tag="xf")
                nc.sync.dma_start(xf[:], xTr[:, :, rs:rs + rblk])

                hb = hbp.tile([P, FK, rblk], bf16, tag="hb")
                for k in range(FK):
                    nc.vector.tensor_copy(hb[:, k], h[:, k])

                for s in range(NUM_STEPS):
                    # layer 1: a = tanh((dt*W1s).T-chunks @ h'b + b1_eff(s))
                    a = apool.tile([P, FK, rblk], bf16, tag="a")
                    for m in range(FK):
                        ps = psp.tile([P, rblk], f32, tag="ps")
                        for k in range(FK):
                            mm_group(ps, w1, k, m, hb, k, k == 0, k == FK - 1)
                        col = s * FK + m
                        nc.scalar.activation(a[:, m], ps[:], Act.Tanh,
                                             bias=b1[:, col:col + 1])
                    # layer 2: deriv = tanh(W2.T-chunks @ a + b2)
                    d = dpool.tile([P, FK, rblk], bf16, tag="d")
                    for m in range(FK):
                        ps = psp.tile([P, rblk], f32, tag="ps")
                        for k in range(FK):
                            mm_group(ps, w2, k, m, a, k, k == 0, k == FK - 1)
                        nc.scalar.activation(d[:, m], ps[:], Act.Tanh,
                                             bias=b2[:, m:m + 1])
                    # h' += deriv ; refresh bf16 mirror
                    hb = hbp.tile([P, FK, rblk], bf16, tag="hb")
                    for k in range(FK):
                        nc.vector.tensor_tensor(h[:, k], h[:, k], d[:, k], Alu.add)
                        nc.vector.tensor_copy(hb[:, k], h[:, k])

                # gate + combine, per output feature chunk
                for m in range(FK):
                    ps = psp.tile([P, rblk], f32, tag="ps")
                    for k in range(FK):
                        mm_group(ps, wg, k, m, xb, k, k == 0, False)
                    for k in range(FK):
                        mm_group(ps, wg, FK + k, m, hb, k, False, k == FK - 1)
                    g = gp.tile([P, rblk], f32, tag="g")
                    nc.scalar.activation(g[:], ps[:], Act.Sigmoid,
                                         bias=bg[:, m:m + 1])
                    ev = scp.tile([P, rblk], f32, tag="sc")
                    nc.vector.tensor_scalar_mul(ev[:], h[:, m], float(DT))
                    dif = scp.tile([P, rblk], f32, tag="sc")
                    nc.vector.tensor_tensor(dif[:], ev[:], xf[:, m], Alu.subtract)
                    gd = scp.tile([P, rblk], f32, tag="sc")
                    nc.vector.tensor_tensor(gd[:], g[:], dif[:], Alu.mult)
                    ot = scp.tile([P, rblk], f32, tag="sc")
                    nc.vector.tensor_tensor(ot[:], xf[:, m], gd[:], Alu.add)
                    nc.sync.dma_start(outTr[:, m, rs:rs + rblk], ot[:])

    nc.compile()
    return nc


def _get_nc(npc, rblk=1024):
    key = (npc, rblk)
    if key not in _CACHE:
        _CACHE[key] = _build(npc, rblk)
    return _CACHE[key]


def _host_prep(W1, b1, W2, b2, Wg, bg):
    bf = ml_dtypes.bfloat16
    W1 = np.asarray(W1, np.float32)
    W2 = np.asarray(W2, np.float32)
    Wg = np.asarray(Wg, np.float32)
    b1 = np.asarray(b1, np.float32)
    b2 = np.asarray(b2, np.float32)
    bg = np.asarray(bg, np.float32)

    w1t = np.ascontiguousarray((DT * W1[:, :D]).T).astype(bf)      # [in, out]
    w2t = np.ascontiguousarray(W2.T).astype(bf)
    wgt = np.ascontiguousarray(
        np.concatenate([Wg[:, :D].T, DT * Wg[:, D:].T], axis=0)).astype(bf)

    ts = (DT * np.arange(NUM_STEPS)).astype(np.float32)
    b1r = b1.reshape(FK, P)                                        # [m, p]
    wtr = np.ascontiguousarray(W1[:, D]).reshape(FK, P)            # [m, p]
    b1e = b1r[None, :, :] + ts[:, None, None] * wtr[None, :, :]    # [s, m, p]
    b1e = np.ascontiguousarray(b1e.transpose(2, 0, 1).reshape(P, NUM_STEPS * FK))
    b2c = np.ascontiguousarray(b2.reshape(FK, P).T)
    bgc = np.ascontiguousarray(bg.reshape(FK, P).T)
    return dict(w1t=w1t, w2t=w2t, wgt=wgt,
                b1e=b1e.astype(np.float32),
                b2c=b2c.astype(np.float32), bgc=bgc.astype(np.float32))


def kernel(current_node_features, previous_hidden_state, W1, b1, W2, b2, Wg, bg):
    from concourse.bass_utils import run_bass_kernel_spmd

    bf = ml_dtypes.bfloat16
    x = np.asarray(current_node_features, np.float32)
    h0 = np.asarray(previous_hidden_state, np.float32)
    weights = _host_prep(W1, b1, W2, b2, Wg, bg)

    inv_dt = np.float32(1.0 / DT)
    in_maps = []
    for c in range(NCORES):
        sl = slice(c * NPC, (c + 1) * NPC)
        xTc = np.ascontiguousarray(x[sl].T)
        in_maps.append(dict(
            hT=np.ascontiguousarray(h0[sl].T) * inv_dt,
            xT=xTc,
            xTb=xTc.astype(bf),
            **weights,
        ))

    nc = _get_nc(NPC)
    trace = bool(os.environ.get("BASS_TRACE"))
    res = run_bass_kernel_spmd(nc, in_maps, core_ids=list(range(NCORES)),
                               trace=trace)
    LAST["res"] = res

    out = np.empty((N_TOTAL, D), np.float32)
    for c in range(NCORES):
        out[c * NPC:(c + 1) * NPC] = res.results[c]["outT"].T
    return out, out
